# revision 15
# baseline (speedup 1.0000x reference)
"""GQA (softermax) Trainium2 kernel, tensor-parallel over kv-head groups.

Problem: x[1,2048,4096], 32 q-heads / 8 kv-heads, head_dim=128, base-2
softmax (softermax), fp32 reference. Each of the 8 cores owns one kv-head
group (4 q-heads, 512 q dims, 128 kv dims) and computes a partial
o-projection [2048,4096]; the host sums the 8 partials and adds o_b.

v3 (bf16, SBUF-layout DMA): all matmuls bf16 (1 cyc/row, FWL weight
loads). The host pre-transposes/pre-chunks x and weights into the exact
SBUF tile layouts so every DMA descriptor is an 8-32KB contiguous run
per partition (the naive [E,S] layout gave ~1KB descriptors and only
~5 GB/s per DMA engine). Softmax denominators: DVE bf16 chunk-add chain
(2x packed mode) + one all-ones [128,128] stationary matmul that both
partition-reduces and broadcasts Z; 1/Z via reciprocal_approx_fast.
Exp runs on paired score chunks (FD=1024 spanning 2 PSUM banks) to
amortize ACT per-call overhead.

Per-core dataflow:
  proj:  xT[e,s] (DMA, host-chunked) ; K^T,V^T,Q^T = W^T.T @ xT
         V natural via 16 PE transposes of V^T
  attn:  S^T[k,q] = KT_chunk.T @ QT (pairs of chunks into one 2-bank
         PSUM tile); P^T = exp(S^T * ln2/sqrt(128)) [ACT, FD=1024]
         O^T[d,q] = sum_k Vn_chunk.T @ P^T_chunk   (PSUM accum)
         acc = bf16 chunk-sum of P^T [DVE]; Zb = ones128.T @ acc (PE)
         OTb = O^T * recip_approx(Zb)              [DVE]
  oproj: out[s,e] = sum_h OTb_h_chunk.T @ owT_h    (partial; host sums)
"""

import math
from contextlib import ExitStack

import numpy as np
import ml_dtypes

import concourse.bass as bass
from concourse import bacc
import concourse.mybir as mybir
import concourse.tile as tile
from concourse.bass_utils import run_bass_kernel_spmd
from concourse.masks import make_identity

E = 4096          # embed dim
S = 2048          # sequence
D = 128           # head dim
NHL = 4           # q heads per core
DQ = NHL * D      # 512 q dims per core
DKV = 128         # kv dims per core (1 kv head)
NCORES = 8
NE = E // 128     # 32 embed chunks

SB = 512          # seq block for projection pass
NSB = S // SB
QS = 512          # q block in attention
NQS = S // QS
NKT = S // 128    # 16 k chunks
NOB = S // 128    # 16 output row blocks
NOE = E // 512    # 8 output col blocks

F32 = mybir.dt.float32
BF = mybir.dt.bfloat16
BF_NP = ml_dtypes.bfloat16
EXP_SCALE = math.log(2.0) / math.sqrt(D)

_CACHED_NC = None


def build_bass():
    nc = bacc.Bacc(None)

    # all inputs pre-chunked on host into SBUF tile layout [p, chunk, cols]
    xt_d = nc.declare_dram_parameter("xt", [NSB, 128, NE, SB], BF,
                                     isOutput=False)
    qwt_d = nc.declare_dram_parameter("qwt", [128, NE, DQ], BF, isOutput=False)
    qb_d = nc.declare_dram_parameter("qb", [DQ], F32, isOutput=False)
    kwt_d = nc.declare_dram_parameter("kwt", [128, NE, DKV], BF, isOutput=False)
    kb_d = nc.declare_dram_parameter("kb", [DKV], F32, isOutput=False)
    vwt_d = nc.declare_dram_parameter("vwt", [128, NE, DKV], BF, isOutput=False)
    vb_d = nc.declare_dram_parameter("vb", [DKV], F32, isOutput=False)
    owt_d = nc.declare_dram_parameter("owt", [128, NHL, E], BF, isOutput=False)
    out_d = nc.declare_dram_parameter("out", [NOB, NOE, 128, 512], BF,
                                      isOutput=True)

    Id = mybir.ActivationFunctionType.Identity
    Exp = mybir.ActivationFunctionType.Exp

    def copy_ps(i, dst, src):
        # alternate psum->sbuf copies between ACT and DVE
        if i % 2 == 0:
            nc.scalar.copy(dst, src)
        else:
            nc.vector.tensor_copy(dst, src)

    with tile.TileContext(nc) as tc, ExitStack() as es:
        consts = es.enter_context(tc.tile_pool(name="consts", bufs=1))
        persist = es.enter_context(tc.tile_pool(name="persist", bufs=1))

        # ---------------- constants ----------------
        ident = consts.tile([128, 128], BF)
        make_identity(nc, ident[:, :])
        ones128 = consts.tile([128, 128], BF)
        nc.gpsimd.memset(ones128[:, :], 1.0)

        qb_sb = consts.tile([128, NHL], F32)
        nc.sync.dma_start(qb_sb[:, :], qb_d[:].rearrange("(t p) -> p t", p=128))
        kb_sb = consts.tile([128, 1], F32)
        nc.sync.dma_start(kb_sb[:, :], kb_d[:].rearrange("(p o) -> p o", o=1))
        vb_sb = consts.tile([128, 1], F32)
        nc.sync.dma_start(vb_sb[:, :], vb_d[:].rearrange("(p o) -> p o", o=1))

        # warm the ACT exp table during the initial DMA wait
        warm = consts.tile([128, 1], F32)
        nc.scalar.activation(warm[:, :], kb_sb[:, 0:1], Exp, scale=1.0)

        # ---------------- persistent tensors (24 KB/partition) ----------
        KT = persist.tile([128, S], BF)              # K^T [d, seq]
        QT = persist.tile([128, NHL, S], BF)         # Q^T per head [d, seq]
        Vn = persist.tile([128, NKT, 128], BF)       # V natural [seq, d] chunks

        # ================= phase 1: projections =================
        with (
            tc.tile_pool(name="xz", bufs=2) as xz,
            tc.tile_pool(name="wproj", bufs=1) as wproj,
            tc.tile_pool(name="vtmp", bufs=2) as vtmp,
            tc.tile_pool(name="ps_acc", bufs=3, space="PSUM") as ps_acc,
            tc.tile_pool(name="ps_tr", bufs=4, space="PSUM") as ps_tr,
        ):
            kwT = wproj.tile([128, NE, DKV], BF, tag="kwT")
            for g in range(4):
                # e-group split: the first K matmuls only need group 0, so
                # compute starts while the rest of the weights stream in
                nc.sync.dma_start(kwT[:, g * 8:(g + 1) * 8, :],
                                  kwt_d[:, g * 8:(g + 1) * 8, :])
            vwT = wproj.tile([128, NE, DKV], BF, tag="vwT")
            for g in range(4):
                nc.sync.dma_start(vwT[:, g * 8:(g + 1) * 8, :],
                                  vwt_d[:, g * 8:(g + 1) * 8, :])
            qwT = wproj.tile([128, NE, DQ], BF, tag="qwT")

            for sb in range(NSB):
                xTb = xz.tile([128, NE, SB], BF, tag="x")
                if sb == 0:
                    # split the first block by columns AND e-groups so the
                    # K projection starts as soon as the first slices land
                    for g in range(4):
                        gs = slice(g * 8, (g + 1) * 8)
                        nc.sync.dma_start(xTb[:, gs, 0:256],
                                          xt_d[0, :, gs, 0:256])
                    nc.sync.dma_start(xTb[:, :, 256:SB],
                                      xt_d[0, :, :, 256:SB])
                    nc.sync.dma_start(qwT[:, :, :], qwt_d[:, :, :])
                    splits = [(0, 256), (256, SB)]
                else:
                    nc.sync.dma_start(xTb[:, :, :], xt_d[sb, :, :, :])
                    splits = [(0, SB)]

                for c0, c1 in splits:
                    w = c1 - c0
                    ssl = slice(sb * SB + c0, sb * SB + c1)
                    ps_k = ps_acc.tile([128, SB], F32, tag="acc")
                    for e in range(NE):
                        nc.tensor.matmul(ps_k[:, 0:w], kwT[:, e, :],
                                         xTb[:, e, c0:c1],
                                         start=(e == 0), stop=(e == NE - 1))
                    nc.scalar.activation(KT[:, ssl], ps_k[:, 0:w], Id,
                                         bias=kb_sb[:, 0:1])

                    ps_v = ps_acc.tile([128, SB], F32, tag="acc")
                    for e in range(NE):
                        nc.tensor.matmul(ps_v[:, 0:w], vwT[:, e, :],
                                         xTb[:, e, c0:c1],
                                         start=(e == 0), stop=(e == NE - 1))
                    VTb = vtmp.tile([128, SB], BF, tag="vt")
                    nc.scalar.activation(VTb[:, 0:w], ps_v[:, 0:w], Id,
                                         bias=vb_sb[:, 0:1])
                    for i in range(w // 128):
                        t = (sb * SB + c0) // 128 + i
                        tp = ps_tr.tile([128, 128], BF, tag="tr")
                        nc.tensor.transpose(
                            tp[:, :], VTb[:, i * 128:(i + 1) * 128],
                            ident[:, :])
                        nc.vector.tensor_copy(Vn[:, t, :], tp[:, :])

                for c0, c1 in splits:
                    w = c1 - c0
                    ssl = slice(sb * SB + c0, sb * SB + c1)
                    for h in range(NHL):
                        ps_q = ps_acc.tile([128, SB], F32, tag="acc")
                        for e in range(NE):
                            nc.tensor.matmul(
                                ps_q[:, 0:w],
                                qwT[:, e, h * 128:(h + 1) * 128],
                                xTb[:, e, c0:c1],
                                start=(e == 0), stop=(e == NE - 1))
                        nc.scalar.activation(QT[:, h, ssl], ps_q[:, 0:w], Id,
                                             bias=qb_sb[:, h:h + 1])

        # ================= phase 2: attention + fused o-proj =================
        with (
            tc.tile_pool(name="wo", bufs=1) as wo,
            tc.tile_pool(name="attn", bufs=2) as attn,
            tc.tile_pool(name="obp", bufs=3) as obp,
            tc.tile_pool(name="ps_s", bufs=2, space="PSUM") as ps_s,
            tc.tile_pool(name="ps_o", bufs=2, space="PSUM") as ps_o,
            tc.tile_pool(name="ps_po", bufs=2, space="PSUM") as ps_po,
        ):
            owT = wo.tile([128, NHL, E], BF, tag="owT")
            nc.sync.dma_start(owT[:, :, :], owt_d[:, :, :])

            def emit_oproj(qi, OTb):
                for sl in range(QS // 128):
                    blk = qi * (QS // 128) + sl
                    for ec in range(NOE):
                        po = ps_po.tile([128, 512], F32, tag="po")
                        for dh in range(NHL):
                            nc.tensor.matmul(
                                po[:, :],
                                OTb[:, dh, sl * 128:(sl + 1) * 128],
                                owT[:, dh, ec * 512:(ec + 1) * 512],
                                start=(dh == 0), stop=(dh == NHL - 1))
                        ob = obp.tile([128, 512], BF, tag="ob")
                        # DVE-only: keeps ACT free for the exp stream
                        nc.vector.tensor_copy(ob[:, :], po[:, :])
                        nc.sync.dma_start(out_d[blk, ec, :, :], ob[:, :])

            pending = None
            for qi in range(NQS):
                qsl = slice(qi * QS, (qi + 1) * QS)
                OTb = attn.tile([128, NHL, QS], BF, tag="OTb")
                for h in range(NHL):
                    PT = attn.tile([128, NKT, QS], BF, tag="PT")
                    for kp in range(NKT // 2):
                        sps = ps_s.tile([128, 2, QS], F32, tag="s")
                        for j in range(2):
                            kt = kp * 2 + j
                            nc.tensor.matmul(sps[:, j, :],
                                             KT[:, kt * 128:(kt + 1) * 128],
                                             QT[:, h, qsl],
                                             start=True, stop=True)
                        # exp over both chunks in one ACT call (FD=1024)
                        nc.scalar.activation(PT[:, kp * 2:kp * 2 + 2, :],
                                             sps[:, :, :], Exp,
                                             scale=EXP_SCALE)
                    ops = ps_o.tile([128, QS], F32, tag="o")
                    for kt in range(NKT):
                        nc.tensor.matmul(ops[:, :], Vn[:, kt, :], PT[:, kt, :],
                                         start=(kt == 0), stop=(kt == NKT - 1))
                    # Z: bf16 chunk-sum on DVE, then one all-ones matmul that
                    # partition-reduces AND broadcasts Z to all 128 rows
                    acc = attn.tile([128, QS], BF, tag="acc")
                    nc.vector.tensor_add(acc[:, :], PT[:, 0, :], PT[:, 1, :])
                    for kt in range(2, NKT):
                        nc.vector.tensor_add(acc[:, :], acc[:, :], PT[:, kt, :])
                    zps = ps_po.tile([128, QS], F32, tag="po")
                    nc.tensor.matmul(zps[:, :], ones128[:, :], acc[:, :],
                                     start=True, stop=True)
                    bcs = attn.tile([128, QS], F32, tag="bcs")
                    nc.vector.reciprocal_approx_fast(bcs[:, :], zps[:, :])
                    nc.vector.tensor_mul(OTb[:, h, :], ops[:, :], bcs[:, :])
                emit_oproj(qi, OTb)

    nc.finalize()
    return nc


def make_in_maps(x, q_w, q_b, k_w, k_b, v_w, v_b, o_w):
    x2 = np.asarray(x, np.float32).reshape(S, E)
    # xt[sb, p, g, sl] = x[sb*SB+sl, g*128+p]
    xt = np.ascontiguousarray(
        x2.T.reshape(NE, 128, NSB, SB).transpose(2, 1, 0, 3)).astype(BF_NP)
    q_w = np.asarray(q_w, np.float32)
    k_w = np.asarray(k_w, np.float32)
    v_w = np.asarray(v_w, np.float32)
    o_w = np.asarray(o_w, np.float32)
    in_maps = []
    for c in range(NCORES):
        qsl = slice(c * DQ, (c + 1) * DQ)
        ksl = slice(c * DKV, (c + 1) * DKV)
        # w^T [E, dout] chunked to [p, g, dout]
        qwt = q_w[qsl].T.reshape(NE, 128, DQ).transpose(1, 0, 2)
        kwt = k_w[ksl].T.reshape(NE, 128, DKV).transpose(1, 0, 2)
        vwt = v_w[ksl].T.reshape(NE, 128, DKV).transpose(1, 0, 2)
        # o_w slice^T [DQ, E] chunked to [p, h, E]
        owt = o_w[:, qsl].T.reshape(NHL, 128, E).transpose(1, 0, 2)
        in_maps.append({
            "xt": xt,
            "qwt": np.ascontiguousarray(qwt).astype(BF_NP),
            "qb": np.ascontiguousarray(np.asarray(q_b, np.float32)[qsl]),
            "kwt": np.ascontiguousarray(kwt).astype(BF_NP),
            "kb": np.ascontiguousarray(np.asarray(k_b, np.float32)[ksl]),
            "vwt": np.ascontiguousarray(vwt).astype(BF_NP),
            "vb": np.ascontiguousarray(np.asarray(v_b, np.float32)[ksl]),
            "owt": np.ascontiguousarray(owt).astype(BF_NP),
        })
    return in_maps


def kernel(x, q_w, q_b, k_w, k_b, v_w, v_b, o_w, o_b):
    global _CACHED_NC
    in_maps = make_in_maps(x, q_w, q_b, k_w, k_b, v_w, v_b, o_w)
    if _CACHED_NC is None:
        _CACHED_NC = build_bass()
    res = run_bass_kernel_spmd(_CACHED_NC, in_maps, list(range(NCORES)))
    out = np.zeros((S, E), np.float64)
    for i in range(NCORES):
        o = res.results[i]["out"].astype(np.float32)
        out += o.transpose(0, 2, 1, 3).reshape(S, E).astype(np.float64)
    out += np.asarray(o_b, np.float64)
    return out.astype(np.float32).reshape(1, S, E)


# revision 16
# speedup vs baseline: 1.0150x; 1.0150x over previous
"""GQA (softermax) Trainium2 kernel, tensor-parallel over kv-head groups.

Problem: x[1,2048,4096], 32 q-heads / 8 kv-heads, head_dim=128, base-2
softmax (softermax), fp32 reference. Each of the 8 cores owns one kv-head
group (4 q-heads, 512 q dims, 128 kv dims) and computes a partial
o-projection [2048,4096]; the host sums the 8 partials and adds o_b.

v3 (bf16, SBUF-layout DMA): all matmuls bf16 (1 cyc/row, FWL weight
loads). The host pre-transposes/pre-chunks x and weights into the exact
SBUF tile layouts so every DMA descriptor is an 8-32KB contiguous run
per partition (the naive [E,S] layout gave ~1KB descriptors and only
~5 GB/s per DMA engine). Softmax denominators: DVE bf16 chunk-add chain
(2x packed mode) + one all-ones [128,128] stationary matmul that both
partition-reduces and broadcasts Z; 1/Z via reciprocal_approx_fast.
Exp runs on paired score chunks (FD=1024 spanning 2 PSUM banks) to
amortize ACT per-call overhead.

Per-core dataflow:
  proj:  xT[e,s] (DMA, host-chunked) ; K^T,V^T,Q^T = W^T.T @ xT
         V natural via 16 PE transposes of V^T
  attn:  S^T[k,q] = KT_chunk.T @ QT (pairs of chunks into one 2-bank
         PSUM tile); P^T = exp(S^T * ln2/sqrt(128)) [ACT, FD=1024]
         O^T[d,q] = sum_k Vn_chunk.T @ P^T_chunk   (PSUM accum)
         acc = bf16 chunk-sum of P^T [DVE]; Zb = ones128.T @ acc (PE)
         OTb = O^T * recip_approx(Zb)              [DVE]
  oproj: out[s,e] = sum_h OTb_h_chunk.T @ owT_h    (partial; host sums)
"""

import math
from contextlib import ExitStack

import numpy as np
import ml_dtypes

import concourse.bass as bass
from concourse import bacc
import concourse.mybir as mybir
import concourse.tile as tile
from concourse.bass_utils import run_bass_kernel_spmd
from concourse.masks import make_identity

E = 4096          # embed dim
S = 2048          # sequence
D = 128           # head dim
NHL = 4           # q heads per core
DQ = NHL * D      # 512 q dims per core
DKV = 128         # kv dims per core (1 kv head)
NCORES = 8
NE = E // 128     # 32 embed chunks

SB = 512          # seq block for projection pass
NSB = S // SB
QS = 512          # q block in attention
NQS = S // QS
NKT = S // 128    # 16 k chunks
NOB = S // 128    # 16 output row blocks
NOE = E // 512    # 8 output col blocks

F32 = mybir.dt.float32
BF = mybir.dt.bfloat16
BF_NP = ml_dtypes.bfloat16
EXP_SCALE = math.log(2.0) / math.sqrt(D)

_CACHED_NC = None


def build_bass():
    nc = bacc.Bacc(None)

    # all inputs pre-chunked on host into SBUF tile layout [p, chunk, cols]
    xt_d = nc.declare_dram_parameter("xt", [NSB, 128, NE, SB], BF,
                                     isOutput=False)
    qwt_d = nc.declare_dram_parameter("qwt", [128, NE, DQ], BF, isOutput=False)
    qb_d = nc.declare_dram_parameter("qb", [DQ], F32, isOutput=False)
    kwt_d = nc.declare_dram_parameter("kwt", [128, NE, DKV], BF, isOutput=False)
    kb_d = nc.declare_dram_parameter("kb", [DKV], F32, isOutput=False)
    vwt_d = nc.declare_dram_parameter("vwt", [128, NE, DKV], BF, isOutput=False)
    vb_d = nc.declare_dram_parameter("vb", [DKV], F32, isOutput=False)
    owt_d = nc.declare_dram_parameter("owt", [128, NHL, E], BF, isOutput=False)
    out_d = nc.declare_dram_parameter("out", [NOB, NOE, 128, 512], BF,
                                      isOutput=True)

    Id = mybir.ActivationFunctionType.Identity
    Exp = mybir.ActivationFunctionType.Exp

    def copy_ps(i, dst, src):
        # alternate psum->sbuf copies between ACT and DVE
        if i % 2 == 0:
            nc.scalar.copy(dst, src)
        else:
            nc.vector.tensor_copy(dst, src)

    with tile.TileContext(nc) as tc, ExitStack() as es:
        consts = es.enter_context(tc.tile_pool(name="consts", bufs=1))
        persist = es.enter_context(tc.tile_pool(name="persist", bufs=1))

        # ---------------- constants ----------------
        ident = consts.tile([128, 128], BF)
        make_identity(nc, ident[:, :])
        ones128 = consts.tile([128, 128], BF)
        nc.gpsimd.memset(ones128[:, :], 1.0)

        qb_sb = consts.tile([128, NHL], F32)
        nc.sync.dma_start(qb_sb[:, :], qb_d[:].rearrange("(t p) -> p t", p=128))
        kb_sb = consts.tile([128, 1], F32)
        nc.sync.dma_start(kb_sb[:, :], kb_d[:].rearrange("(p o) -> p o", o=1))
        vb_sb = consts.tile([128, 1], F32)
        nc.sync.dma_start(vb_sb[:, :], vb_d[:].rearrange("(p o) -> p o", o=1))

        # warm the ACT exp table during the initial DMA wait
        warm = consts.tile([128, 1], F32)
        nc.scalar.activation(warm[:, :], kb_sb[:, 0:1], Exp, scale=1.0)

        # ---------------- persistent tensors (24 KB/partition) ----------
        KT = persist.tile([128, S], BF)              # K^T [d, seq]
        QT = persist.tile([128, NHL, S], BF)         # Q^T per head [d, seq]
        Vn = persist.tile([128, NKT, 128], BF)       # V natural [seq, d] chunks

        # ================= phase 1: projections =================
        with (
            tc.tile_pool(name="xz", bufs=2) as xz,
            tc.tile_pool(name="wproj", bufs=1) as wproj,
            tc.tile_pool(name="vtmp", bufs=2) as vtmp,
            tc.tile_pool(name="ps_acc", bufs=3, space="PSUM") as ps_acc,
            tc.tile_pool(name="ps_tr", bufs=4, space="PSUM") as ps_tr,
        ):
            kwT = wproj.tile([128, NE, DKV], BF, tag="kwT")
            for g in range(4):
                # e-group split: the first K matmuls only need group 0, so
                # compute starts while the rest of the weights stream in
                nc.sync.dma_start(kwT[:, g * 8:(g + 1) * 8, :],
                                  kwt_d[:, g * 8:(g + 1) * 8, :])
            vwT = wproj.tile([128, NE, DKV], BF, tag="vwT")
            for g in range(4):
                nc.sync.dma_start(vwT[:, g * 8:(g + 1) * 8, :],
                                  vwt_d[:, g * 8:(g + 1) * 8, :])
            qwT = wproj.tile([128, NE, DQ], BF, tag="qwT")

            for sb in range(NSB):
                xTb = xz.tile([128, NE, SB], BF, tag="x")
                if sb == 0:
                    # split the first block by columns AND e-groups so the
                    # K projection starts as soon as the first slices land
                    for g in range(4):
                        gs = slice(g * 8, (g + 1) * 8)
                        nc.sync.dma_start(xTb[:, gs, 0:256],
                                          xt_d[0, :, gs, 0:256])
                    nc.sync.dma_start(xTb[:, :, 256:SB],
                                      xt_d[0, :, :, 256:SB])
                    nc.sync.dma_start(qwT[:, :, :], qwt_d[:, :, :])
                    splits = [(0, 256), (256, SB)]
                else:
                    nc.sync.dma_start(xTb[:, :, :], xt_d[sb, :, :, :])
                    splits = [(0, SB)]

                for c0, c1 in splits:
                    w = c1 - c0
                    ssl = slice(sb * SB + c0, sb * SB + c1)
                    ps_k = ps_acc.tile([128, SB], F32, tag="acc")
                    for e in range(NE):
                        nc.tensor.matmul(ps_k[:, 0:w], kwT[:, e, :],
                                         xTb[:, e, c0:c1],
                                         start=(e == 0), stop=(e == NE - 1))
                    nc.scalar.activation(KT[:, ssl], ps_k[:, 0:w], Id,
                                         bias=kb_sb[:, 0:1])

                    ps_v = ps_acc.tile([128, SB], F32, tag="acc")
                    for e in range(NE):
                        nc.tensor.matmul(ps_v[:, 0:w], vwT[:, e, :],
                                         xTb[:, e, c0:c1],
                                         start=(e == 0), stop=(e == NE - 1))
                    VTb = vtmp.tile([128, SB], BF, tag="vt")
                    nc.scalar.activation(VTb[:, 0:w], ps_v[:, 0:w], Id,
                                         bias=vb_sb[:, 0:1])
                    for i in range(w // 128):
                        t = (sb * SB + c0) // 128 + i
                        tp = ps_tr.tile([128, 128], BF, tag="tr")
                        nc.tensor.transpose(
                            tp[:, :], VTb[:, i * 128:(i + 1) * 128],
                            ident[:, :])
                        nc.vector.tensor_copy(Vn[:, t, :], tp[:, :])

                for c0, c1 in splits:
                    w = c1 - c0
                    ssl = slice(sb * SB + c0, sb * SB + c1)
                    for h in range(NHL):
                        ps_q = ps_acc.tile([128, SB], F32, tag="acc")
                        for e in range(NE):
                            nc.tensor.matmul(
                                ps_q[:, 0:w],
                                qwT[:, e, h * 128:(h + 1) * 128],
                                xTb[:, e, c0:c1],
                                start=(e == 0), stop=(e == NE - 1))
                        nc.scalar.activation(QT[:, h, ssl], ps_q[:, 0:w], Id,
                                             bias=qb_sb[:, h:h + 1])

        # ================= phase 2: attention + fused o-proj =================
        with (
            tc.tile_pool(name="wo", bufs=1) as wo,
            tc.tile_pool(name="attn", bufs=2) as attn,
            tc.tile_pool(name="obp", bufs=3) as obp,
            tc.tile_pool(name="ps_s", bufs=2, space="PSUM") as ps_s,
            tc.tile_pool(name="ps_o", bufs=2, space="PSUM") as ps_o,
            tc.tile_pool(name="ps_po", bufs=2, space="PSUM") as ps_po,
        ):
            owT = wo.tile([128, NHL, E], BF, tag="owT")
            nc.sync.dma_start(owT[:, :, :], owt_d[:, :, :])

            def emit_oproj(qi, OTb):
                for sl in range(QS // 128):
                    blk = qi * (QS // 128) + sl
                    for ec in range(NOE):
                        po = ps_po.tile([128, 512], F32, tag="po")
                        for dh in range(NHL):
                            nc.tensor.matmul(
                                po[:, :],
                                OTb[:, dh, sl * 128:(sl + 1) * 128],
                                owT[:, dh, ec * 512:(ec + 1) * 512],
                                start=(dh == 0), stop=(dh == NHL - 1))
                        ob = obp.tile([128, 512], BF, tag="ob")
                        copy_ps(sl + ec, ob[:, :], po[:, :])
                        nc.sync.dma_start(out_d[blk, ec, :, :], ob[:, :])

            pending = None
            for qi in range(NQS):
                qsl = slice(qi * QS, (qi + 1) * QS)
                OTb = attn.tile([128, NHL, QS], BF, tag="OTb")
                for h in range(NHL):
                    PT = attn.tile([128, NKT, QS], BF, tag="PT")
                    for kp in range(NKT // 2):
                        sps = ps_s.tile([128, 2, QS], F32, tag="s")
                        for j in range(2):
                            kt = kp * 2 + j
                            nc.tensor.matmul(sps[:, j, :],
                                             KT[:, kt * 128:(kt + 1) * 128],
                                             QT[:, h, qsl],
                                             start=True, stop=True)
                        # exp over both chunks in one ACT call (FD=1024)
                        nc.scalar.activation(PT[:, kp * 2:kp * 2 + 2, :],
                                             sps[:, :, :], Exp,
                                             scale=EXP_SCALE)
                    ops = ps_o.tile([128, QS], F32, tag="o")
                    for kt in range(NKT):
                        nc.tensor.matmul(ops[:, :], Vn[:, kt, :], PT[:, kt, :],
                                         start=(kt == 0), stop=(kt == NKT - 1))
                    # Z: bf16 chunk-sum on DVE, then one all-ones matmul that
                    # partition-reduces AND broadcasts Z to all 128 rows
                    acc = attn.tile([128, QS], BF, tag="acc")
                    nc.vector.tensor_add(acc[:, :], PT[:, 0, :], PT[:, 1, :])
                    for kt in range(2, NKT):
                        nc.vector.tensor_add(acc[:, :], acc[:, :], PT[:, kt, :])
                    zps = ps_po.tile([128, QS], F32, tag="po")
                    nc.tensor.matmul(zps[:, :], ones128[:, :], acc[:, :],
                                     start=True, stop=True)
                    bcs = attn.tile([128, QS], F32, tag="bcs")
                    nc.vector.reciprocal_approx_fast(bcs[:, :], zps[:, :])
                    nc.vector.tensor_mul(OTb[:, h, :], ops[:, :], bcs[:, :])
                emit_oproj(qi, OTb)

    nc.finalize()
    return nc


def make_in_maps(x, q_w, q_b, k_w, k_b, v_w, v_b, o_w):
    x2 = np.asarray(x, np.float32).reshape(S, E)
    # xt[sb, p, g, sl] = x[sb*SB+sl, g*128+p]
    xt = np.ascontiguousarray(
        x2.T.reshape(NE, 128, NSB, SB).transpose(2, 1, 0, 3)).astype(BF_NP)
    q_w = np.asarray(q_w, np.float32)
    k_w = np.asarray(k_w, np.float32)
    v_w = np.asarray(v_w, np.float32)
    o_w = np.asarray(o_w, np.float32)
    in_maps = []
    for c in range(NCORES):
        qsl = slice(c * DQ, (c + 1) * DQ)
        ksl = slice(c * DKV, (c + 1) * DKV)
        # w^T [E, dout] chunked to [p, g, dout]
        qwt = q_w[qsl].T.reshape(NE, 128, DQ).transpose(1, 0, 2)
        kwt = k_w[ksl].T.reshape(NE, 128, DKV).transpose(1, 0, 2)
        vwt = v_w[ksl].T.reshape(NE, 128, DKV).transpose(1, 0, 2)
        # o_w slice^T [DQ, E] chunked to [p, h, E]
        owt = o_w[:, qsl].T.reshape(NHL, 128, E).transpose(1, 0, 2)
        in_maps.append({
            "xt": xt,
            "qwt": np.ascontiguousarray(qwt).astype(BF_NP),
            "qb": np.ascontiguousarray(np.asarray(q_b, np.float32)[qsl]),
            "kwt": np.ascontiguousarray(kwt).astype(BF_NP),
            "kb": np.ascontiguousarray(np.asarray(k_b, np.float32)[ksl]),
            "vwt": np.ascontiguousarray(vwt).astype(BF_NP),
            "vb": np.ascontiguousarray(np.asarray(v_b, np.float32)[ksl]),
            "owt": np.ascontiguousarray(owt).astype(BF_NP),
        })
    return in_maps


def kernel(x, q_w, q_b, k_w, k_b, v_w, v_b, o_w, o_b):
    global _CACHED_NC
    in_maps = make_in_maps(x, q_w, q_b, k_w, k_b, v_w, v_b, o_w)
    if _CACHED_NC is None:
        _CACHED_NC = build_bass()
    res = run_bass_kernel_spmd(_CACHED_NC, in_maps, list(range(NCORES)))
    out = np.zeros((S, E), np.float64)
    for i in range(NCORES):
        o = res.results[i]["out"].astype(np.float32)
        out += o.transpose(0, 2, 1, 3).reshape(S, E).astype(np.float64)
    out += np.asarray(o_b, np.float64)
    return out.astype(np.float32).reshape(1, S, E)
